# revision 26
# baseline (speedup 1.0000x reference)
"""Trainium2 Bass kernel for nn_CopyMechanism (v5).

Math (per batch b, one NeuronCore per batch):
  out[g,c] = softmax_c(mask ? (score_h[g]+score_c[c]) : -inf)
             * sigmoid(gate_h[g]+gate_c[c]+b0)

softmax_c of (score_h[g]+score_c[c]) == softmax_c(score_c) (score_h constant
along c), so copy_probs is g-independent and w_attn[:H] drops out.
encoder_output is unused by the reference. Scores are O(1): exp needs no max
subtraction; masking is additive (sc - 1e5 -> exp underflows to 0).

Pipeline (per core = one batch; 8 ctx chunks of 512 rows, macro-steps of 2):
  - ctx chunk DMA (SWDGE f32->bf16): partition p <- 4 consecutive rows
    4p..4p+3 => 128 x 16KB-src descriptors per 2MB chunk.
  - Transposes on the f32 bitcast view (one [128,128]-f32 transpose moves a
    [128,256]-bf16 block, 16/chunk); the row permutation is undone by a
    stride-4 PSUM out-AP. Evacuation copies split across DVE/ACT.
  - ONE combined dot-matmul set: stationary [128,128] = [wg x96 | wa x32]
    replicated columns, so 8 matmuls/chunk produce gc broadcast on
    partitions 0-95 and sc on partitions 96-127 of one PSUM tile [128,1024]
    (two chunks side by side). The mask row (pre-broadcast to partitions
    96-127 by a stride-0 DMA) is added on DVE into an SBUF staging tile
    that feeds Exp.
  - ALL activations in the exp_and_others set (one table load):
    sigmoid via tanh: q = (tanh(.5x)+1) * e * (0.5/Z).
    Per macro (2 chunks): 1x Exp [32,1024] -> e fp16 + fused f32 Z accum;
    6x Tanh [96|32, 1024] (g-tiles of 96, bias 0.5*(gh+b0) per partition);
    DVE: (t+1) tensor_scalar + q=(t+1)*e_b tensor_tensor, fp16.
  - Consumer stage lags one macro-step behind the producer so PE never
    waits on ACT/DVE.
  - Tail: q * (0.5/Z) tensor_scalar fp32-out, 12x ~0.75MB HWDGE out DMAs.
"""
import sys

if "/opt/trn_rl_repo" not in sys.path:
    sys.path.insert(0, "/opt/trn_rl_repo")

import numpy as np
from contextlib import ExitStack

B, G, C, H = 8, 512, 4096, 1024
N_CORES = 8
P = 128
CJ = C // 512          # 8 ctx chunks of 512 rows
NM = CJ // 2           # 4 macro steps of 2 chunks
NJ = 4                 # h2 (f32-pair) blocks of 128 per 1024-h
GT = 96                # g-tile height (last tile is 32)
NT = 6                 # number of g-tiles: 5x96 + 1x32

_cache = {}


def _g_tiles():
    out = []
    for i in range(NT):
        g0 = i * GT
        rows = GT if i < NT - 1 else G - GT * (NT - 1)
        out.append((i, g0, rows))
    return out


def _build():
    import concourse.bass as bass
    import concourse.tile as tile
    from concourse import bacc, mybir
    from concourse.masks import make_identity

    f32 = mybir.dt.float32
    bf16 = mybir.dt.bfloat16
    fp16 = mybir.dt.float16
    ts = bass.ts
    Act = mybir.ActivationFunctionType

    nc = bacc.Bacc("TRN2", target_bir_lowering=False, debug=False,
                   num_devices=N_CORES)
    hid_d = nc.dram_tensor("hid", [G, H], f32, kind="ExternalInput").ap()
    ctx_d = nc.dram_tensor("ctx", [C, H], f32, kind="ExternalInput").ap()
    madd_d = nc.dram_tensor("madd", [1, C], f32, kind="ExternalInput").ap()
    # combined dot weights: per (J,pi) a [128,128] block with cols 0-95 =
    # wg_c (gc broadcast), cols 96-127 = wa_c (sc), parity-split over h
    wmatC_d = nc.dram_tensor("wmatC", [P, 8 * P], f32,
                             kind="ExternalInput").ap()
    # parity-split hid weights [128, (J,pi)] (wg_h)
    whid_d = nc.dram_tensor("whid", [P, 8], f32, kind="ExternalInput").ap()
    bg_d = nc.dram_tensor("bg", [1, 1], f32, kind="ExternalInput").ap()
    out_d = nc.dram_tensor("out", [G, C], f32, kind="ExternalOutput").ap()

    with tile.TileContext(nc) as tc:
        with ExitStack() as ctx:
            singles = ctx.enter_context(tc.tile_pool(name="singles", bufs=1))
            ctxp = ctx.enter_context(tc.tile_pool(name="ctxp", bufs=3))
            hidp = ctx.enter_context(tc.tile_pool(name="hidp", bufs=1))
            ctp = ctx.enter_context(tc.tile_pool(name="ctp", bufs=2))
            htp = ctx.enter_context(tc.tile_pool(name="htp", bufs=1))
            ebp = ctx.enter_context(tc.tile_pool(name="ebp", bufs=2))
            erp = ctx.enter_context(tc.tile_pool(name="erp", bufs=2))
            scmp = ctx.enter_context(tc.tile_pool(name="scmp", bufs=2))
            tp_ = ctx.enter_context(tc.tile_pool(name="tp_", bufs=2))
            t2p = ctx.enter_context(tc.tile_pool(name="t2p", bufs=3))
            qp = ctx.enter_context(tc.tile_pool(name="qp", bufs=1))
            outp = ctx.enter_context(tc.tile_pool(name="outp", bufs=2))
            smp = ctx.enter_context(tc.tile_pool(name="smp", bufs=1))
            tp_ps = ctx.enter_context(
                tc.tile_pool(name="tp_ps", bufs=2, space="PSUM"))
            dt_ps = ctx.enter_context(
                tc.tile_pool(name="dt_ps", bufs=1, space="PSUM"))
            gs_ps_p = ctx.enter_context(
                tc.tile_pool(name="gs_ps", bufs=2, space="PSUM"))

            ctx4s = []

            def emit_ctx_dma(j):
                ctx4 = ctxp.tile([P, 4, H], bf16, tag="ctx4")
                nc.gpsimd.dma_start(
                    out=ctx4,
                    in_=ctx_d[j * 512:(j + 1) * 512, :].rearrange(
                        "(p ci) h -> p ci h", p=P))
                ctx4s.append(ctx4)

            # small cast-DMAs first on the SWDGE queue
            wmatC_b = singles.tile([P, 8 * P], bf16)
            nc.gpsimd.dma_start(out=wmatC_b, in_=wmatC_d)
            whid_b = singles.tile([P, 8], bf16)
            nc.gpsimd.dma_start(out=whid_b, in_=whid_d)
            # mask row: cast to partition 0, then DRE-broadcast to all
            madd_r = singles.tile([1, C], bf16)
            nc.gpsimd.dma_start(out=madd_r, in_=madd_d)
            maddF = singles.tile([P, C], bf16)
            nc.gpsimd.partition_broadcast(maddF, madd_r)
            bg_col = singles.tile([P, 1], f32)
            nc.gpsimd.dma_start(
                out=bg_col,
                in_=bass.AP(tensor=bg_d.tensor, offset=bg_d.offset,
                            ap=[[0, P], [1, 1]]))

            emit_ctx_dma(0)
            emit_ctx_dma(1)
            hid4 = hidp.tile([P, 4, H], bf16, tag="hid4")
            nc.gpsimd.dma_start(
                out=hid4,
                in_=hid_d.rearrange("(p ci) h -> p ci h", p=P))
            emit_ctx_dma(2)


            ident_f = singles.tile([P, P], f32)
            make_identity(nc, ident_f)
            ones_f = singles.tile([1, P], f32)
            nc.gpsimd.memset(ones_f[:, :], 1.0)

            # ---- persistent tiles ----
            z32 = smp.tile([P, NM], f32)      # rows 96-127 used
            ghh = smp.tile([GT, NT], f32)     # bias table at base 0
            q = [qp.tile([P, C], fp16, tag=f"q{i}", name=f"q{i}")
                 for i in range(NT)]

            def transp_block(src4, ctxT, ch_slot):
                """16 f32-packed transposes of one chunk -> ctxT[:,:,slot]"""
                src_f = src4[:, :, :].bitcast(f32)    # [P, 4, 512]
                for J in range(NJ):
                    tp = tp_ps.tile([P, 512], f32, tag="tps")
                    tpv = tp[:, :].rearrange("p (c ci) -> p ci c", ci=4)
                    for ci in range(4):
                        nc.tensor.transpose(tpv[:, ci, :],
                                            src_f[:, ci, ts(J, P)], ident_f)
                    if J in (0, 2):
                        nc.vector.tensor_copy(ctxT[:, J, ch_slot, :], tp)
                    else:
                        nc.scalar.copy(ctxT[:, J, ch_slot, :], tp)

            def producer(jj):
                """macro jj = chunks 2jj, 2jj+1 -> gcsc PSUM + e_b"""
                ctxT = ctp.tile([P, NJ, 2, 512], f32, tag="ctxT")
                transp_block(ctx4s[2 * jj], ctxT, 0)
                transp_block(ctx4s[2 * jj + 1], ctxT, 1)
                for jn in (2 * jj + 3, 2 * jj + 4):
                    if jn < CJ:
                        emit_ctx_dma(jn)

                gs_ps = gs_ps_p.tile([P, 1024], f32, tag="gs")
                for ch in range(2):
                    half = gs_ps[:, ch * 512:(ch + 1) * 512]
                    for J in range(NJ):
                        cv = ctxT[:, J, ch, :].bitcast(bf16).rearrange(
                            "p (c two) -> p two c", two=2)
                        for pi in range(2):
                            nc.tensor.matmul(
                                half, wmatC_b[:, ts(J * 2 + pi, P)],
                                cv[:, pi, :],
                                start=(J == 0 and pi == 0),
                                stop=(J == NJ - 1 and pi == 1))

                # masked scores (full width, base 0; only rows 96-127 are
                # sc — the rest is junk that only feeds unused e rows)
                scm = scmp.tile([P, 1024], f32, tag="scm")
                nc.vector.tensor_add(
                    scm, gs_ps,
                    maddF[:, 2 * jj * 512:(2 * jj + 2) * 512])
                e_sb = erp.tile([P, 1024], fp16, tag="e_sb")
                nc.scalar.activation(e_sb, scm, Act.Exp,
                                     accum_out=z32[:, jj:jj + 1])
                # hop the true e row (partition 96) to partition 0, then DRE
                e_r0 = erp.tile([1, 1024], fp16, tag="e_r0")
                nc.gpsimd.dma_start(out=e_r0, in_=e_sb[96:97, :])
                e_b = ebp.tile([P, 1024], fp16, tag="e_b")
                nc.gpsimd.partition_broadcast(e_b, e_r0)
                return gs_ps, e_b

            def consumer(jj, gs_ps, e_b):
                for i, g0, rows in _g_tiles():
                    pr = slice(0, rows)
                    t = tp_.tile([P, 1024], fp16, tag="t")
                    nc.scalar.activation(t[pr, :], gs_ps[pr, :], Act.Tanh,
                                         bias=ghh[pr, i:i + 1], scale=0.5)
                    t2 = t2p.tile([P, 1024], fp16, tag="t2")
                    nc.vector.tensor_scalar(out=t2[pr, :], in0=t[pr, :],
                                            scalar1=1.0, scalar2=None,
                                            op0=mybir.AluOpType.add)
                    nc.vector.tensor_mul(
                        q[i][pr, 2 * jj * 512:(2 * jj + 2) * 512],
                        t2[pr, :], e_b[pr, :])

            def hid_prelude():
                hidT = htp.tile([P, NJ, 1, 512], f32, tag="hidT")
                transp_block(hid4, hidT, 0)
                gh_ps = dt_ps.tile([1, 512], f32, tag="ghr")
                for J in range(NJ):
                    hv = hidT[:, J, 0, :].bitcast(bf16).rearrange(
                        "p (c two) -> p two c", two=2)
                    for pi in range(2):
                        nc.tensor.matmul(
                            gh_ps, whid_b[:, J * 2 + pi:J * 2 + pi + 1],
                            hv[:, pi, :],
                            start=(J == 0 and pi == 0),
                            stop=(J == NJ - 1 and pi == 1))
                ghp_sb = smp.tile([1, 512], f32)
                nc.vector.tensor_copy(ghp_sb, gh_ps)
                # bias columns at base 0 via rank-1 matmuls
                ghc_ps = tp_ps.tile([P, 512], f32, tag="tps")
                for i, g0, rows in _g_tiles():
                    nc.tensor.matmul(ghc_ps[0:rows, i:i + 1],
                                     ghp_sb[0:1, g0:g0 + rows],
                                     ones_f[0:1, 0:1], start=True, stop=True)
                nc.vector.tensor_scalar(out=ghh, in0=ghc_ps[0:GT, 0:NT],
                                        scalar1=bg_col[0:GT, 0:1],
                                        scalar2=0.5,
                                        op0=mybir.AluOpType.add,
                                        op1=mybir.AluOpType.mult)

            # ---- software-pipelined macro loop ----
            prev = producer(0)
            hid_prelude()
            for jj in range(1, NM):
                cur = producer(jj)
                consumer(jj - 1, *prev)
                prev = cur
            consumer(NM - 1, *prev)

            # ---- tail: Z, 0.5/Z, final scale + output DMAs ----
            z_r0 = smp.tile([1, NM], f32)
            nc.gpsimd.dma_start(out=z_r0, in_=z32[96:97, :])
            z1 = smp.tile([1, 1], f32)
            nc.vector.reduce_sum(z1, z_r0, axis=mybir.AxisListType.X)
            rz = smp.tile([1, 1], f32)
            nc.vector.reciprocal(rz, z1)
            rzh = smp.tile([1, 1], f32)
            nc.vector.tensor_scalar(out=rzh, in0=rz, scalar1=0.5,
                                    scalar2=None, op0=mybir.AluOpType.mult)
            rz_col = smp.tile([P, 1], f32)
            nc.gpsimd.partition_broadcast(rz_col, rzh)
            CH = C // 2
            for i, g0, rows in _g_tiles():
                pr = slice(0, rows)
                out_t = outp.tile([P, C], f32, tag="out_t")
                for h2 in range(2):
                    fs = slice(h2 * CH, (h2 + 1) * CH)
                    nc.vector.tensor_scalar(out=out_t[pr, fs],
                                            in0=q[i][pr, fs],
                                            scalar1=rz_col[pr, 0:1],
                                            scalar2=None,
                                            op0=mybir.AluOpType.mult)
                    nc.sync.dma_start(out=out_d[g0:g0 + rows, fs],
                                      in_=out_t[pr, fs])

    nc.compile()
    return nc


def _get_nc():
    if "nc" not in _cache:
        _cache["nc"] = _build()
    return _cache["nc"]


def _make_weights(w_attn, w_gate):
    w_attn = np.asarray(w_attn, dtype=np.float32)
    w_gate = np.asarray(w_gate, dtype=np.float32)
    wg_c, wa_c = w_gate[H:], w_attn[H:]
    wg_h = w_gate[:H]
    k = np.arange(P)
    wmatC = np.zeros((P, 8 * P), dtype=np.float32)
    whid = np.zeros((P, 8), dtype=np.float32)
    for J in range(4):
        for pi in range(2):
            h = J * 256 + 2 * k + pi
            blk = np.zeros((P, P), dtype=np.float32)
            blk[:, 0:GT] = wg_c[h][:, None]
            blk[:, GT:P] = wa_c[h][:, None]
            wmatC[:, (J * 2 + pi) * P:(J * 2 + pi + 1) * P] = blk
            whid[:, J * 2 + pi] = wg_h[h]
    return wmatC, whid


def make_in_maps(hidden_states, context_hidden, w_attn, w_gate, b_gate,
                 copy_mask):
    wmatC, whid = _make_weights(w_attn, w_gate)
    bg = np.asarray(b_gate, dtype=np.float32).reshape(1, 1)
    in_maps = []
    for b in range(B):
        madd = np.where(np.asarray(copy_mask[b]) == 0, -1e5, 0.0)
        madd = madd.reshape(1, C).astype(np.float32)
        in_maps.append({
            "hid": np.ascontiguousarray(hidden_states[b], dtype=np.float32),
            "ctx": np.ascontiguousarray(context_hidden[b], dtype=np.float32),
            "madd": np.ascontiguousarray(madd),
            "wmatC": wmatC,
            "whid": whid,
            "bg": bg,
        })
    return in_maps


def kernel(hidden_states, context_hidden, encoder_output, w_attn, w_gate,
           b_gate, copy_mask):
    from concourse.bass_utils import run_bass_kernel_spmd

    nc = _get_nc()
    in_maps = make_in_maps(hidden_states, context_hidden, w_attn, w_gate,
                           b_gate, copy_mask)
    res = run_bass_kernel_spmd(nc, in_maps, core_ids=list(range(N_CORES)))
    return np.stack([res.results[b]["out"] for b in range(B)], axis=0)
